# revision 28
# baseline (speedup 1.0000x reference)
"""Trainium2 Bass kernel: 12-head self-attention block (B=2, N=4096, C=768).

Sharding: token-parallel over the 8192 (batch, token) rows. Core c (0..7)
handles batch c//4, query rows [(c%4)*1024, (c%4+1)*1024). Every core
redundantly computes K/V for its whole batch (zero collectives); the host
rotates each core's token order so its own 1024 query tokens come first
(attention is permutation-invariant over keys).

Phase C is ACT-bound: softmax exponentiates 50M scores per core; each
[128,1024] ACTIVATE measures ~1005ns -> ~386us floor for 384 tiles, and
the phase runs within ~7us of it. The PE must stay under that budget
per key-chunk. Two structural choices make that true:

  * Scores are computed with K=128 matmuls against a zero-padded qT
    (qpad slot h0 has partitions 64:128 zeroed, slot h1 has 0:64 zeroed;
    lhsT is the full 128-partition kT chunk, so the cross-head contraction
    terms multiply by zero). K=64 matmuls would put the PE in 64x128
    tiling mode, and switching tiling modes between score and AV matmuls
    (K=128) drains the PE array -- measured ~90ns per matmul extra. With
    padding everything stays in 128x128 mode and one LDWEIGHTS per key
    chunk feeds all 4 score matmuls. Stream time is N cycles regardless
    of K, so the padded contraction costs nothing.
  * v_aug carries a ones column so the AV matmul emits softmax
    denominators for free (row 64); reciprocal_approx_fast (input
    relocated to partition 0) feeds a gpsimd broadcast and one DVE
    multiply per tile.

All staging in SBUF, bf16 everywhere. fp8 was tested and rejected for
BOTH V (2.1e-2 rel err) and Q/K projections (9.9e-2 on hw, 9.6e-2 in a
numpy sim -- the softmax here is peaky, so score noise does not average
out). Cross-core K/V all-gather was measured and rejected: each
collective_compute on this axon harness has a ~60us entry/exit floor
(only ~20us wire time per 6.3MB chunk), so a pipelined exchange cannot
beat recomputation. Dense back-to-back matmul streaming keeps the PE
HAM clock gate at 2.4 GHz; prologue DMAs are interleaved (wqkv/x chunk
pairs) and memsets run on gpsimd so the first matmul issues at ~13us.

Device pipeline per core:
  phase A (per 1024-token quarter): qT/kT[col,t] = Wqkv[:, :1536].T @ x^T
           (q only for quarter 0 -> qpad halves; k -> kT_sb), v[t,(h,d)] =
           x @ Wqkv[:, 1536:] -> v_sb[t, kc, h, 0:64], ones at col 64.
  phase C: per head-pair hp, key-chunk kc: scoresT[key,q] = kT.T @ qpad_h,
           eT = exp(SCALE*sT) on ACT, av[0:65] += v_aug.T @ eT (two
           iterations behind its exp; et pool holds the slack).
  phase D: out[t,c] = tokensT.T @ Wproj + bproj.
"""

import sys

import numpy as np

try:
    import concourse  # noqa: F401
except ImportError:  # pragma: no cover
    sys.path.insert(0, "/opt/trn_rl_repo")

import ml_dtypes

import concourse.bass as bass  # noqa: F401
import concourse.mybir as mybir
import concourse.tile as tile
from concourse import bacc
from concourse.bass_utils import run_bass_kernel_spmd

B, N, C = 2, 4096, 768
H, D = 12, 64
NT = 1024  # query tokens per core
SCALE = float(D) ** -0.5
NCORES = 8
KC = N // 128  # 32 key chunks per batch
VW = D + 1  # v_aug row width per head: [v(64), ones]

F32 = mybir.dt.float32
BF16 = mybir.dt.bfloat16
EXP = mybir.ActivationFunctionType.Exp
MUL = mybir.AluOpType.mult
ADD = mybir.AluOpType.add


def build_graph():
    nc = bacc.Bacc(
        "TRN2", target_bir_lowering=False, debug=False, num_devices=NCORES
    )

    xT_e = nc.declare_dram_parameter("xT", [C, N], BF16, isOutput=False)
    wqkv_e = nc.declare_dram_parameter("Wqkv", [C, 3 * C], BF16, isOutput=False)
    wproj_e = nc.declare_dram_parameter("Wproj", [C, C], BF16, isOutput=False)
    bproj_e = nc.declare_dram_parameter("bproj", [1, C], F32, isOutput=False)
    out_e = nc.declare_dram_parameter("out", [NT, C], F32, isOutput=True)

    with tile.TileContext(nc) as tc:
        _build_body(nc, tc, xT_e, wqkv_e, wproj_e, bproj_e, out_e)
    nc.finalize()
    return nc


def _build_body(nc, tc, xT_e, wqkv_e, wproj_e, bproj_e, out_e):
    with tc.tile_pool(name="persist", bufs=1) as persist:
        # ---- persistent SBUF ----
        # qpad[:, hp, h, :]: head h of pair hp at partitions h*64:(h+1)*64,
        # other 64 partitions stay zero (enables K=128 score matmuls).
        qpad = persist.tile([128, 6, 2, NT], BF16, tag="qpad")
        kT_sb = persist.tile([128, 6, N], BF16, tag="kT")
        # v_aug[token_part, key_chunk, head, 0:64]=v, [.,.,.,64]=1.0
        v_sb = persist.tile([128, KC, H, VW], BF16, tag="v")
        tokT = [
            persist.tile([128, NT], BF16, tag=f"tokT{i}", name=f"tokT{i}")
            for i in range(6)
        ]
        # phase D weights, DMA'd off the prologue critical path
        wproj_sb = persist.tile([128, 6, C], BF16, tag="wproj")
        bproj_sb = persist.tile([1, C], F32, tag="bproj")
        bproj_bc = persist.tile([128, C], F32, tag="bproj_bc")

        # memsets on gpsimd: keeps the DVE queue free for phase A copies
        # (only the pad halves of qpad need zeroing; phase A writes the rest)
        nc.gpsimd.memset(qpad[64:128, :, 0, :], 0.0)
        nc.gpsimd.memset(qpad[0:64, :, 1, :], 0.0)
        # ones column of v_aug only (strided memset); v writes fill 0:64
        nc.gpsimd.memset(v_sb[:, :, :, D : D + 1], 1.0)

        # preload the exp ACT table set during phase A so phase C's first
        # activation doesn't pay the ~2.7us ACT_TABLE_LOAD + drain
        warm = persist.tile([1, 16], F32, tag="warm")
        nc.vector.memset(warm[:], 0.0)
        nc.scalar.activation(warm[:], warm[:], EXP, scale=1.0)

        # ================= phase A: qkv projection =================
        with (
            tc.tile_pool(name="pa_w", bufs=1) as paw,
            tc.tile_pool(name="pa_x", bufs=2) as pax,
            tc.tile_pool(name="pa_psum", bufs=2, space="PSUM") as pap,
        ):
            wqkv_sb = paw.tile([128, 6, 3 * C], BF16, tag="wqkv")
            xq0 = pax.tile([128, 6, NT], BF16, tag="xq")
            # interleave weight/activation chunk DMAs so the first matmul's
            # operands (wqkv kc0 + xq kc0) land first
            # wave 1: just the cc0 weight columns + quarter-0 x, so the
            # first q matmul group is fed ~5us earlier; wave 2: the rest
            for kc in range(6):
                nc.sync.dma_start(
                    wqkv_sb[:, kc, 0:128], wqkv_e[kc * 128 : (kc + 1) * 128, 0:128]
                )
                nc.sync.dma_start(
                    xq0[:, kc, :], xT_e[kc * 128 : (kc + 1) * 128, 0:NT]
                )
            for kc in range(6):
                nc.sync.dma_start(
                    wqkv_sb[:, kc, 128:],
                    wqkv_e[kc * 128 : (kc + 1) * 128, 128:],
                )

            for tq in range(4):  # 1024-token quarters
                tq0 = tq * NT
                if tq == 0:
                    xq = xq0
                else:
                    xq = pax.tile([128, 6, NT], BF16, tag="xq")
                    for kc in range(6):
                        nc.sync.dma_start(
                            xq[:, kc, :],
                            xT_e[kc * 128 : (kc + 1) * 128, tq0 : tq0 + NT],
                        )

                # q (quarter 0 only) + k columns, transposed orientation.
                # Both token halves share each weight chunk (one LDW, 2 MMs).
                ccs = range(12) if tq == 0 else range(6, 12)
                for cc in ccs:
                    pj = pap.tile([128, 1024], F32, tag="pj")
                    for kc in range(6):
                        for th in range(2):
                            nc.tensor.matmul(
                                pj[:, th * 512 : (th + 1) * 512],
                                wqkv_sb[:, kc, cc * 128 : (cc + 1) * 128],
                                xq[:, kc, th * 512 : (th + 1) * 512],
                                start=(kc == 0),
                                stop=(kc == 5),
                            )
                    if cc < 6:
                        # split into per-head halves of qpad; pad stays 0
                        nc.vector.tensor_copy(
                            qpad[0:64, cc, 0, :], pj[0:64, :]
                        )
                        nc.vector.tensor_copy(
                            qpad[64:128, cc, 1, :], pj[64:128, :]
                        )
                    else:
                        nc.vector.tensor_copy(
                            kT_sb[:, cc - 6, tq0 : tq0 + NT], pj[:]
                        )

                # v columns (token-major); one xq LDW feeds both col halves
                for tcn in range(8):
                    kcn = tq * 8 + tcn  # global 128-token (=key) chunk
                    pj = pap.tile([128, 768], F32, tag="pjv")
                    for kc in range(6):
                        for c0, c1 in ((0, 512), (512, 768)):
                            nc.tensor.matmul(
                                pj[:, c0:c1],
                                xq[:, kc, tcn * 128 : (tcn + 1) * 128],
                                wqkv_sb[:, kc, 2 * C + c0 : 2 * C + c1],
                                start=(kc == 0),
                                stop=(kc == 5),
                            )
                    nc.vector.tensor_copy(
                        v_sb[:, kcn, :, 0:D],
                        pj[:].rearrange("p (h d) -> p h d", d=D),
                    )

        # phase D weights: needed only at the end
        nc.sync.dma_start(bproj_sb[:], bproj_e[:])
        nc.gpsimd.partition_broadcast(bproj_bc[:], bproj_sb[:])
        for cc in range(6):
            nc.sync.dma_start(
                wproj_sb[:, cc, :], wproj_e[cc * 128 : (cc + 1) * 128, :]
            )

        # ================= phase C: attention =================
        # Per kc: one kT LDW + 4 K=128 score MMs (zero-padded q), 2 exps
        # on ACT, 4 K=128 AV MMs two iterations behind (et pool holds the
        # slack so PE never waits on ACT).
        # PSUM: sc 2x[128,1024] (2 banks each) + av 4x[65,512] = 8 banks.
        with (
            tc.tile_pool(name="et_pool", bufs=12) as etp,
            tc.tile_pool(name="sc_pool", bufs=2, space="PSUM") as scp,
            tc.tile_pool(name="av_pool", bufs=4, space="PSUM") as avp,
            tc.tile_pool(name="small", bufs=4) as smp,
        ):
            for hp in range(6):
                avs = []
                for i in range(4):
                    avt = avp.tile([65, 512], F32, tag="av", name=f"av_{hp}_{i}")
                    avs.append(avt)

                # AV for kc runs 2 iterations behind its exp
                def do_av(kc, e0, e1):
                    for hd, et in ((0, e0), (1, e1)):
                        for qh in range(2):
                            jsl = slice(qh * 512, (qh + 1) * 512)
                            nc.tensor.matmul(
                                avs[2 * hd + qh][:],
                                v_sb[:, kc, 2 * hp + hd, :],
                                et[:, jsl],
                                start=(kc == 0),
                                stop=(kc == KC - 1),
                            )

                pend = []
                for kc in range(KC):
                    ksl = slice(kc * 128, (kc + 1) * 128)
                    sc0 = scp.tile([128, 1024], F32, tag="sc")
                    sc1 = scp.tile([128, 1024], F32, tag="sc")
                    # full-K matmuls: one LDW (kT chunk) serves all four;
                    # the zero pad in qpad kills the cross-head terms
                    for qh in range(2):
                        qsl = slice(qh * 512, (qh + 1) * 512)
                        nc.tensor.matmul(
                            sc0[:, qsl],
                            kT_sb[:, hp, ksl],
                            qpad[:, hp, 0, qsl],
                            start=True,
                            stop=True,
                        )
                    for qh in range(2):
                        qsl = slice(qh * 512, (qh + 1) * 512)
                        nc.tensor.matmul(
                            sc1[:, qsl],
                            kT_sb[:, hp, ksl],
                            qpad[:, hp, 1, qsl],
                            start=True,
                            stop=True,
                        )
                    e0 = etp.tile([128, 1024], BF16, tag="et")
                    e1 = etp.tile([128, 1024], BF16, tag="et")
                    nc.scalar.activation(e0[:], sc0[:], EXP, scale=SCALE)
                    nc.scalar.activation(e1[:], sc1[:], EXP, scale=SCALE)
                    pend.append((kc, e0, e1))
                    if len(pend) > 2:
                        do_av(*pend.pop(0))
                for p in pend:
                    do_av(*p)

                for i, av in enumerate(avs):
                    hd, qh = i // 2, i % 2
                    qsl = slice(qh * 512, (qh + 1) * 512)
                    # one copy releases the PSUM bank; the recip chain
                    # then runs off the PE critical path
                    av_sb = smp.tile([65, 512], F32, tag="av_sb")
                    nc.vector.tensor_copy(av_sb[:], av[:])
                    # relocate denominator row to partition 0 (1-partition
                    # DVE copies cross quadrants; the custom approx-recip op
                    # needs all operands co-resident at partition 0)
                    den = smp.tile([1, 512], F32, tag="den")
                    nc.vector.tensor_copy(den[:], av_sb[64:65, :])
                    rec = smp.tile([1, 512], F32, tag="rec")
                    nc.vector.reciprocal_approx_fast(rec[:], den[:])
                    bc = smp.tile([64, 512], F32, tag="bc")
                    nc.gpsimd.partition_broadcast(bc[:], rec[:])
                    if hd == 0:
                        nc.vector.tensor_tensor(
                            out=tokT[hp][0:64, qsl],
                            in0=av_sb[0:64, :],
                            in1=bc[:],
                            op=MUL,
                        )
                    else:
                        tmp = smp.tile([64, 512], BF16, tag="tmp")
                        nc.vector.tensor_tensor(
                            out=tmp[:], in0=av_sb[0:64, :], in1=bc[:], op=MUL
                        )
                        # partition-shifting copy (base 0 -> 64) via DMA
                        nc.sync.dma_start(tokT[hp][64:128, qsl], tmp[:])

        # ================= phase D: output projection =================
        with (
            tc.tile_pool(name="pd_psum", bufs=4, space="PSUM") as pdp,
            tc.tile_pool(name="pd_sbuf", bufs=4) as pds,
        ):
            for tcn in range(8):
                pj = pdp.tile([128, 768], F32, tag="pd")
                for cc in range(6):
                    for c0, c1 in ((0, 512), (512, 768)):
                        nc.tensor.matmul(
                            pj[:, c0:c1],
                            tokT[cc][:, tcn * 128 : (tcn + 1) * 128],
                            wproj_sb[:, cc, c0:c1],
                            start=(cc == 0),
                            stop=(cc == 5),
                        )
                ot = pds.tile([128, 768], F32, tag="ot")
                nc.vector.tensor_tensor(
                    out=ot[:], in0=pj[:], in1=bproj_bc[:], op=ADD
                )
                nc.sync.dma_start(out_e[tcn * 128 : (tcn + 1) * 128, :], ot[:])


_CACHE = {}


def _get_graph():
    if "nc" not in _CACHE:
        _CACHE["nc"] = build_graph()
    return _CACHE["nc"]


def make_in_maps(x, W_qkv, W_proj, b_proj):
    x = np.asarray(x, dtype=np.float32)
    W_qkv = np.asarray(W_qkv, dtype=np.float32).astype(ml_dtypes.bfloat16)
    W_proj = np.asarray(W_proj, dtype=np.float32).astype(ml_dtypes.bfloat16)
    b_proj = np.asarray(b_proj, dtype=np.float32).reshape(1, C)
    W_qkv = np.ascontiguousarray(W_qkv)
    W_proj = np.ascontiguousarray(W_proj)
    in_maps = []
    for c in range(NCORES):
        bb, r0 = c // 4, (c % 4) * NT
        idx = np.r_[r0 : r0 + NT, 0:r0, r0 + NT : N]
        xT = np.ascontiguousarray(
            x[bb][idx].T.astype(ml_dtypes.bfloat16)
        )  # own tokens first
        in_maps.append(
            {
                "xT": xT,
                "Wqkv": W_qkv,
                "Wproj": W_proj,
                "bproj": b_proj,
            }
        )
    return in_maps


def run(x, W_qkv, W_proj, b_proj, trace=False):
    nc = _get_graph()
    in_maps = make_in_maps(x, W_qkv, W_proj, b_proj)
    res = run_bass_kernel_spmd(
        nc, in_maps, core_ids=list(range(NCORES)), trace=trace
    )
    out = np.zeros((B, N, C), dtype=np.float32)
    for c in range(NCORES):
        bb, r0 = c // 4, (c % 4) * NT
        out[bb, r0 : r0 + NT, :] = res.results[c]["out"]
    return out, res


def kernel(x, W_qkv, W_proj, b_proj):
    out, _ = run(x, W_qkv, W_proj, b_proj, trace=False)
    return out


# revision 29
# speedup vs baseline: 1.0097x; 1.0097x over previous
"""Trainium2 Bass kernel: 12-head self-attention block (B=2, N=4096, C=768).

Sharding: token-parallel over the 8192 (batch, token) rows. Core c (0..7)
handles batch c//4, query rows [(c%4)*1024, (c%4+1)*1024). Every core
redundantly computes K/V for its whole batch (zero collectives); the host
rotates each core's token order so its own 1024 query tokens come first
(attention is permutation-invariant over keys).

Phase C is ACT-bound: softmax exponentiates 50M scores per core; each
[128,1024] ACTIVATE measures ~1005ns -> ~386us floor for 384 tiles, and
the phase runs within ~7us of it. The PE must stay under that budget
per key-chunk. Two structural choices make that true:

  * Scores are computed with K=128 matmuls against a zero-padded qT
    (qpad slot h0 has partitions 64:128 zeroed, slot h1 has 0:64 zeroed;
    lhsT is the full 128-partition kT chunk, so the cross-head contraction
    terms multiply by zero). K=64 matmuls would put the PE in 64x128
    tiling mode, and switching tiling modes between score and AV matmuls
    (K=128) drains the PE array -- measured ~90ns per matmul extra. With
    padding everything stays in 128x128 mode and one LDWEIGHTS per key
    chunk feeds all 4 score matmuls. Stream time is N cycles regardless
    of K, so the padded contraction costs nothing.
  * v_aug carries a ones column so the AV matmul emits softmax
    denominators for free (row 64); reciprocal_approx_fast (input
    relocated to partition 0) feeds a gpsimd broadcast and one DVE
    multiply per tile.

All staging in SBUF, bf16 everywhere. fp8 was tested and rejected for
BOTH V (2.1e-2 rel err) and Q/K projections (9.9e-2 on hw, 9.6e-2 in a
numpy sim -- the softmax here is peaky, so score noise does not average
out). Cross-core K/V all-gather was measured and rejected: each
collective_compute on this axon harness has a ~60us entry/exit floor
(only ~20us wire time per 6.3MB chunk), so a pipelined exchange cannot
beat recomputation. Dense back-to-back matmul streaming keeps the PE
HAM clock gate at 2.4 GHz; prologue DMAs are interleaved (wqkv/x chunk
pairs) and memsets run on gpsimd so the first matmul issues at ~13us.

Device pipeline per core:
  phase A (per 1024-token quarter): qT/kT[col,t] = Wqkv[:, :1536].T @ x^T
           (q only for quarter 0 -> qpad halves; k -> kT_sb), v[t,(h,d)] =
           x @ Wqkv[:, 1536:] -> v_sb[t, kc, h, 0:64], ones at col 64.
  phase C: per head-pair hp, key-chunk kc: scoresT[key,q] = kT.T @ qpad_h,
           eT = exp(SCALE*sT) on ACT, av[0:65] += v_aug.T @ eT (two
           iterations behind its exp; et pool holds the slack).
  phase D: out[t,c] = tokensT.T @ Wproj + bproj.
"""

import sys

import numpy as np

try:
    import concourse  # noqa: F401
except ImportError:  # pragma: no cover
    sys.path.insert(0, "/opt/trn_rl_repo")

import ml_dtypes

import concourse.bass as bass  # noqa: F401
import concourse.mybir as mybir
import concourse.tile as tile
from concourse import bacc
from concourse.bass_utils import run_bass_kernel_spmd

B, N, C = 2, 4096, 768
H, D = 12, 64
NT = 1024  # query tokens per core
SCALE = float(D) ** -0.5
NCORES = 8
KC = N // 128  # 32 key chunks per batch
VW = D + 1  # v_aug row width per head: [v(64), ones]

F32 = mybir.dt.float32
BF16 = mybir.dt.bfloat16
EXP = mybir.ActivationFunctionType.Exp
MUL = mybir.AluOpType.mult
ADD = mybir.AluOpType.add


def build_graph():
    nc = bacc.Bacc(
        "TRN2", target_bir_lowering=False, debug=False, num_devices=NCORES
    )

    xT_e = nc.declare_dram_parameter("xT", [C, N], BF16, isOutput=False)
    wqkv_e = nc.declare_dram_parameter("Wqkv", [C, 3 * C], BF16, isOutput=False)
    wproj_e = nc.declare_dram_parameter("Wproj", [C, C], BF16, isOutput=False)
    bproj_e = nc.declare_dram_parameter("bproj", [1, C], F32, isOutput=False)
    out_e = nc.declare_dram_parameter("out", [NT, C], F32, isOutput=True)

    with tile.TileContext(nc) as tc:
        _build_body(nc, tc, xT_e, wqkv_e, wproj_e, bproj_e, out_e)
    nc.finalize()
    return nc


def _build_body(nc, tc, xT_e, wqkv_e, wproj_e, bproj_e, out_e):
    with tc.tile_pool(name="persist", bufs=1) as persist:
        # ---- persistent SBUF ----
        # qpad[:, hp, h, :]: head h of pair hp at partitions h*64:(h+1)*64,
        # other 64 partitions stay zero (enables K=128 score matmuls).
        qpad = persist.tile([128, 6, 2, NT], BF16, tag="qpad")
        kT_sb = persist.tile([128, 6, N], BF16, tag="kT")
        # v_aug[token_part, key_chunk, head, 0:64]=v, [.,.,.,64]=1.0
        v_sb = persist.tile([128, KC, H, VW], BF16, tag="v")
        tokT = [
            persist.tile([128, NT], BF16, tag=f"tokT{i}", name=f"tokT{i}")
            for i in range(6)
        ]
        # phase D weights, DMA'd off the prologue critical path
        wproj_sb = persist.tile([128, 6, C], BF16, tag="wproj")
        bproj_sb = persist.tile([1, C], F32, tag="bproj")
        bproj_bc = persist.tile([128, C], F32, tag="bproj_bc")

        # memsets on gpsimd: keeps the DVE queue free for phase A copies
        # (only the pad halves of qpad need zeroing; phase A writes the rest)
        nc.gpsimd.memset(qpad[64:128, :, 0, :], 0.0)
        nc.gpsimd.memset(qpad[0:64, :, 1, :], 0.0)
        # ones column of v_aug only (strided memset); v writes fill 0:64
        nc.gpsimd.memset(v_sb[:, :, :, D : D + 1], 1.0)

        # preload the exp ACT table set during phase A so phase C's first
        # activation doesn't pay the ~2.7us ACT_TABLE_LOAD + drain
        warm = persist.tile([1, 16], F32, tag="warm")
        nc.vector.memset(warm[:], 0.0)
        nc.scalar.activation(warm[:], warm[:], EXP, scale=1.0)

        # ================= phase A: qkv projection =================
        with (
            tc.tile_pool(name="pa_w", bufs=1) as paw,
            tc.tile_pool(name="pa_x", bufs=2) as pax,
            tc.tile_pool(name="pa_psum", bufs=2, space="PSUM") as pap,
        ):
            wqkv_sb = paw.tile([128, 6, 3 * C], BF16, tag="wqkv")
            xq0 = pax.tile([128, 6, NT], BF16, tag="xq")
            # interleave weight/activation chunk DMAs so the first matmul's
            # operands (wqkv kc0 + xq kc0) land first
            for kc in range(6):
                nc.sync.dma_start(
                    wqkv_sb[:, kc, :], wqkv_e[kc * 128 : (kc + 1) * 128, :]
                )
                nc.sync.dma_start(
                    xq0[:, kc, :], xT_e[kc * 128 : (kc + 1) * 128, 0:NT]
                )

            for tq in range(4):  # 1024-token quarters
                tq0 = tq * NT
                if tq == 0:
                    xq = xq0
                else:
                    xq = pax.tile([128, 6, NT], BF16, tag="xq")
                    for kc in range(6):
                        nc.sync.dma_start(
                            xq[:, kc, :],
                            xT_e[kc * 128 : (kc + 1) * 128, tq0 : tq0 + NT],
                        )

                # q (quarter 0 only) + k columns, transposed orientation.
                # Both token halves share each weight chunk (one LDW, 2 MMs).
                ccs = range(12) if tq == 0 else range(6, 12)
                for cc in ccs:
                    pj = pap.tile([128, 1024], F32, tag="pj")
                    for kc in range(6):
                        for th in range(2):
                            nc.tensor.matmul(
                                pj[:, th * 512 : (th + 1) * 512],
                                wqkv_sb[:, kc, cc * 128 : (cc + 1) * 128],
                                xq[:, kc, th * 512 : (th + 1) * 512],
                                start=(kc == 0),
                                stop=(kc == 5),
                            )
                    if cc < 6:
                        # split into per-head halves of qpad; pad stays 0
                        nc.vector.tensor_copy(
                            qpad[0:64, cc, 0, :], pj[0:64, :]
                        )
                        nc.vector.tensor_copy(
                            qpad[64:128, cc, 1, :], pj[64:128, :]
                        )
                    else:
                        nc.vector.tensor_copy(
                            kT_sb[:, cc - 6, tq0 : tq0 + NT], pj[:]
                        )

                # v columns (token-major); one xq LDW feeds both col halves
                for tcn in range(8):
                    kcn = tq * 8 + tcn  # global 128-token (=key) chunk
                    pj = pap.tile([128, 768], F32, tag="pjv")
                    for kc in range(6):
                        for c0, c1 in ((0, 512), (512, 768)):
                            nc.tensor.matmul(
                                pj[:, c0:c1],
                                xq[:, kc, tcn * 128 : (tcn + 1) * 128],
                                wqkv_sb[:, kc, 2 * C + c0 : 2 * C + c1],
                                start=(kc == 0),
                                stop=(kc == 5),
                            )
                    nc.vector.tensor_copy(
                        v_sb[:, kcn, :, 0:D],
                        pj[:].rearrange("p (h d) -> p h d", d=D),
                    )

        # phase D weights: needed only at the end
        nc.sync.dma_start(bproj_sb[:], bproj_e[:])
        nc.gpsimd.partition_broadcast(bproj_bc[:], bproj_sb[:])
        for cc in range(6):
            nc.sync.dma_start(
                wproj_sb[:, cc, :], wproj_e[cc * 128 : (cc + 1) * 128, :]
            )

        # ================= phase C: attention =================
        # Per kc: one kT LDW + 4 K=128 score MMs (zero-padded q), 2 exps
        # on ACT, 4 K=128 AV MMs two iterations behind (et pool holds the
        # slack so PE never waits on ACT).
        # PSUM: sc 2x[128,1024] (2 banks each) + av 4x[65,512] = 8 banks.
        with (
            tc.tile_pool(name="et_pool", bufs=12) as etp,
            tc.tile_pool(name="sc_pool", bufs=2, space="PSUM") as scp,
            tc.tile_pool(name="av_pool", bufs=4, space="PSUM") as avp,
            tc.tile_pool(name="small", bufs=4) as smp,
        ):
            for hp in range(6):
                avs = []
                for i in range(4):
                    avt = avp.tile([65, 512], F32, tag="av", name=f"av_{hp}_{i}")
                    avs.append(avt)

                # AV for kc runs 2 iterations behind its exp
                def do_av(kc, e0, e1):
                    for hd, et in ((0, e0), (1, e1)):
                        for qh in range(2):
                            jsl = slice(qh * 512, (qh + 1) * 512)
                            nc.tensor.matmul(
                                avs[2 * hd + qh][:],
                                v_sb[:, kc, 2 * hp + hd, :],
                                et[:, jsl],
                                start=(kc == 0),
                                stop=(kc == KC - 1),
                            )

                pend = []
                for kc in range(KC):
                    ksl = slice(kc * 128, (kc + 1) * 128)
                    sc0 = scp.tile([128, 1024], F32, tag="sc")
                    sc1 = scp.tile([128, 1024], F32, tag="sc")
                    # full-K matmuls: one LDW (kT chunk) serves all four;
                    # the zero pad in qpad kills the cross-head terms
                    for qh in range(2):
                        qsl = slice(qh * 512, (qh + 1) * 512)
                        nc.tensor.matmul(
                            sc0[:, qsl],
                            kT_sb[:, hp, ksl],
                            qpad[:, hp, 0, qsl],
                            start=True,
                            stop=True,
                        )
                    for qh in range(2):
                        qsl = slice(qh * 512, (qh + 1) * 512)
                        nc.tensor.matmul(
                            sc1[:, qsl],
                            kT_sb[:, hp, ksl],
                            qpad[:, hp, 1, qsl],
                            start=True,
                            stop=True,
                        )
                    e0 = etp.tile([128, 1024], BF16, tag="et")
                    e1 = etp.tile([128, 1024], BF16, tag="et")
                    nc.scalar.activation(e0[:], sc0[:], EXP, scale=SCALE)
                    nc.scalar.activation(e1[:], sc1[:], EXP, scale=SCALE)
                    pend.append((kc, e0, e1))
                    if len(pend) > 1:
                        do_av(*pend.pop(0))
                for p in pend:
                    do_av(*p)

                for i, av in enumerate(avs):
                    hd, qh = i // 2, i % 2
                    qsl = slice(qh * 512, (qh + 1) * 512)
                    # one copy releases the PSUM bank; the recip chain
                    # then runs off the PE critical path
                    av_sb = smp.tile([65, 512], F32, tag="av_sb")
                    nc.vector.tensor_copy(av_sb[:], av[:])
                    # relocate denominator row to partition 0 (1-partition
                    # DVE copies cross quadrants; the custom approx-recip op
                    # needs all operands co-resident at partition 0)
                    den = smp.tile([1, 512], F32, tag="den")
                    nc.vector.tensor_copy(den[:], av_sb[64:65, :])
                    rec = smp.tile([1, 512], F32, tag="rec")
                    nc.vector.reciprocal_approx_fast(rec[:], den[:])
                    bc = smp.tile([64, 512], F32, tag="bc")
                    nc.gpsimd.partition_broadcast(bc[:], rec[:])
                    if hd == 0:
                        nc.vector.tensor_tensor(
                            out=tokT[hp][0:64, qsl],
                            in0=av_sb[0:64, :],
                            in1=bc[:],
                            op=MUL,
                        )
                    else:
                        tmp = smp.tile([64, 512], BF16, tag="tmp")
                        nc.vector.tensor_tensor(
                            out=tmp[:], in0=av_sb[0:64, :], in1=bc[:], op=MUL
                        )
                        # partition-shifting copy (base 0 -> 64) via DMA
                        nc.sync.dma_start(tokT[hp][64:128, qsl], tmp[:])

        # ================= phase D: output projection =================
        with (
            tc.tile_pool(name="pd_psum", bufs=4, space="PSUM") as pdp,
            tc.tile_pool(name="pd_sbuf", bufs=4) as pds,
        ):
            for tcn in range(8):
                pj = pdp.tile([128, 768], F32, tag="pd")
                for cc in range(6):
                    for c0, c1 in ((0, 512), (512, 768)):
                        nc.tensor.matmul(
                            pj[:, c0:c1],
                            tokT[cc][:, tcn * 128 : (tcn + 1) * 128],
                            wproj_sb[:, cc, c0:c1],
                            start=(cc == 0),
                            stop=(cc == 5),
                        )
                ot = pds.tile([128, 768], F32, tag="ot")
                nc.vector.tensor_tensor(
                    out=ot[:], in0=pj[:], in1=bproj_bc[:], op=ADD
                )
                nc.sync.dma_start(out_e[tcn * 128 : (tcn + 1) * 128, :], ot[:])


_CACHE = {}


def _get_graph():
    if "nc" not in _CACHE:
        _CACHE["nc"] = build_graph()
    return _CACHE["nc"]


def make_in_maps(x, W_qkv, W_proj, b_proj):
    x = np.asarray(x, dtype=np.float32)
    W_qkv = np.asarray(W_qkv, dtype=np.float32).astype(ml_dtypes.bfloat16)
    W_proj = np.asarray(W_proj, dtype=np.float32).astype(ml_dtypes.bfloat16)
    b_proj = np.asarray(b_proj, dtype=np.float32).reshape(1, C)
    W_qkv = np.ascontiguousarray(W_qkv)
    W_proj = np.ascontiguousarray(W_proj)
    in_maps = []
    for c in range(NCORES):
        bb, r0 = c // 4, (c % 4) * NT
        idx = np.r_[r0 : r0 + NT, 0:r0, r0 + NT : N]
        xT = np.ascontiguousarray(
            x[bb][idx].T.astype(ml_dtypes.bfloat16)
        )  # own tokens first
        in_maps.append(
            {
                "xT": xT,
                "Wqkv": W_qkv,
                "Wproj": W_proj,
                "bproj": b_proj,
            }
        )
    return in_maps


def run(x, W_qkv, W_proj, b_proj, trace=False):
    nc = _get_graph()
    in_maps = make_in_maps(x, W_qkv, W_proj, b_proj)
    res = run_bass_kernel_spmd(
        nc, in_maps, core_ids=list(range(NCORES)), trace=trace
    )
    out = np.zeros((B, N, C), dtype=np.float32)
    for c in range(NCORES):
        bb, r0 = c // 4, (c % 4) * NT
        out[bb, r0 : r0 + NT, :] = res.results[c]["out"]
    return out, res


def kernel(x, W_qkv, W_proj, b_proj):
    out, _ = run(x, W_qkv, W_proj, b_proj, trace=False)
    return out


# revision 32
# speedup vs baseline: 1.0239x; 1.0140x over previous
"""Trainium2 Bass kernel: 12-head self-attention block (B=2, N=4096, C=768).

Sharding: token-parallel over the 8192 (batch, token) rows. Core c (0..7)
handles batch c//4, query rows [(c%4)*1024, (c%4+1)*1024). Every core
redundantly computes K/V for its whole batch (zero collectives); the host
rotates each core's token order so its own 1024 query tokens come first
(attention is permutation-invariant over keys).

Phase C is ACT-bound: softmax exponentiates 50M scores per core; each
[128,1024] ACTIVATE measures ~1005ns -> ~386us floor for 384 tiles, and
the phase runs within ~7us of it. The PE must stay under that budget
per key-chunk. Two structural choices make that true:

  * Scores are computed with K=128 matmuls against a zero-padded qT
    (qpad slot h0 has partitions 64:128 zeroed, slot h1 has 0:64 zeroed;
    lhsT is the full 128-partition kT chunk, so the cross-head contraction
    terms multiply by zero). K=64 matmuls would put the PE in 64x128
    tiling mode, and switching tiling modes between score and AV matmuls
    (K=128) drains the PE array -- measured ~90ns per matmul extra. With
    padding everything stays in 128x128 mode and one LDWEIGHTS per key
    chunk feeds all 4 score matmuls. Stream time is N cycles regardless
    of K, so the padded contraction costs nothing.
  * v_aug carries a ones column so the AV matmul emits softmax
    denominators for free (row 64); reciprocal_approx_fast (input
    relocated to partition 0) feeds a gpsimd broadcast and one DVE
    multiply per tile.

All staging in SBUF, bf16 everywhere. fp8 was tested and rejected for
BOTH V (2.1e-2 rel err) and Q/K projections (9.9e-2 on hw, 9.6e-2 in a
numpy sim -- the softmax here is peaky, so score noise does not average
out). Cross-core K/V all-gather was measured and rejected: each
collective_compute on this axon harness has a ~60us entry/exit floor
(only ~20us wire time per 6.3MB chunk), so a pipelined exchange cannot
beat recomputation. Dense back-to-back matmul streaming keeps the PE
HAM clock gate at 2.4 GHz; prologue DMAs are interleaved (wqkv/x chunk
pairs) and memsets run on gpsimd so the first matmul issues at ~13us.

Device pipeline per core:
  phase A (per 1024-token quarter): qT/kT[col,t] = Wqkv[:, :1536].T @ x^T
           (q only for quarter 0 -> qpad halves; k -> kT_sb), v[t,(h,d)] =
           x @ Wqkv[:, 1536:] -> v_sb[t, kc, h, 0:64], ones at col 64.
  phase C: per head-pair hp, key-chunk kc: scoresT[key,q] = kT.T @ qpad_h,
           eT = exp(SCALE*sT) on ACT, av[0:65] += v_aug.T @ eT (one
           iteration behind its exp; et pool holds the slack).
  phase D: out[t,c] = tokensT.T @ Wproj + bproj.
"""

import sys

import numpy as np

try:
    import concourse  # noqa: F401
except ImportError:  # pragma: no cover
    sys.path.insert(0, "/opt/trn_rl_repo")

import ml_dtypes

import concourse.bass as bass  # noqa: F401
import concourse.mybir as mybir
import concourse.tile as tile
from concourse import bacc
from concourse.bass_utils import run_bass_kernel_spmd

B, N, C = 2, 4096, 768
H, D = 12, 64
NT = 1024  # query tokens per core
SCALE = float(D) ** -0.5
NCORES = 8
KC = N // 128  # 32 key chunks per batch
VW = D + 1  # v_aug row width per head: [v(64), ones]

F32 = mybir.dt.float32
BF16 = mybir.dt.bfloat16
EXP = mybir.ActivationFunctionType.Exp
MUL = mybir.AluOpType.mult
ADD = mybir.AluOpType.add


def build_graph():
    nc = bacc.Bacc(
        "TRN2", target_bir_lowering=False, debug=False, num_devices=NCORES
    )

    xT_e = nc.declare_dram_parameter("xT", [C, N], BF16, isOutput=False)
    wqkv_e = nc.declare_dram_parameter("Wqkv", [C, 3 * C], BF16, isOutput=False)
    wproj_e = nc.declare_dram_parameter("Wproj", [C, C], BF16, isOutput=False)
    bproj_e = nc.declare_dram_parameter("bproj", [1, C], F32, isOutput=False)
    out_e = nc.declare_dram_parameter("out", [NT, C], F32, isOutput=True)

    with tile.TileContext(nc) as tc:
        _build_body(nc, tc, xT_e, wqkv_e, wproj_e, bproj_e, out_e)
    nc.finalize()
    return nc


def _build_body(nc, tc, xT_e, wqkv_e, wproj_e, bproj_e, out_e):
    with tc.tile_pool(name="persist", bufs=1) as persist:
        # ---- persistent SBUF ----
        # qpad[:, hp, h, :]: head h of pair hp at partitions h*64:(h+1)*64,
        # other 64 partitions stay zero (enables K=128 score matmuls).
        qpad = persist.tile([128, 6, 2, NT], BF16, tag="qpad")
        kT_sb = persist.tile([128, 6, N], BF16, tag="kT")
        # v_aug[token_part, key_chunk, head, 0:64]=v, [.,.,.,64]=1.0
        v_sb = persist.tile([128, KC, H, VW], BF16, tag="v")
        tokT = [
            persist.tile([128, NT], BF16, tag=f"tokT{i}", name=f"tokT{i}")
            for i in range(6)
        ]
        # phase D weights, DMA'd off the prologue critical path
        wproj_sb = persist.tile([128, 6, C], BF16, tag="wproj")
        bproj_sb = persist.tile([1, C], F32, tag="bproj")
        bproj_bc = persist.tile([128, C], F32, tag="bproj_bc")

        # memsets on gpsimd: keeps the DVE queue free for phase A copies
        # (only the pad halves of qpad need zeroing; phase A writes the rest)
        nc.gpsimd.memset(qpad[64:128, :, 0, :], 0.0)
        nc.gpsimd.memset(qpad[0:64, :, 1, :], 0.0)
        # ones column of v_aug only (strided memset); v writes fill 0:64
        nc.gpsimd.memset(v_sb[:, :, :, D : D + 1], 1.0)

        # preload the exp ACT table set during phase A so phase C's first
        # activation doesn't pay the ~2.7us ACT_TABLE_LOAD + drain
        warm = persist.tile([1, 16], F32, tag="warm")
        nc.vector.memset(warm[:], 0.0)
        nc.scalar.activation(warm[:], warm[:], EXP, scale=1.0)

        # ================= phase A: qkv projection =================
        with (
            tc.tile_pool(name="pa_w", bufs=1) as paw,
            tc.tile_pool(name="pa_x", bufs=2) as pax,
            tc.tile_pool(name="pa_psum", bufs=2, space="PSUM") as pap,
        ):
            wqkv_sb = paw.tile([128, 6, 3 * C], BF16, tag="wqkv")
            xq0 = pax.tile([128, 6, NT], BF16, tag="xq")
            # interleave weight/activation chunk DMAs so the first matmul's
            # operands (wqkv kc0 + xq kc0) land first
            for kc in range(6):
                nc.sync.dma_start(
                    wqkv_sb[:, kc, :], wqkv_e[kc * 128 : (kc + 1) * 128, :]
                )
                nc.sync.dma_start(
                    xq0[:, kc, :], xT_e[kc * 128 : (kc + 1) * 128, 0:NT]
                )

            for tq in range(4):  # 1024-token quarters
                tq0 = tq * NT
                if tq == 0:
                    xq = xq0
                else:
                    xq = pax.tile([128, 6, NT], BF16, tag="xq")
                    for kc in range(6):
                        nc.sync.dma_start(
                            xq[:, kc, :],
                            xT_e[kc * 128 : (kc + 1) * 128, tq0 : tq0 + NT],
                        )

                # q (quarter 0 only) + k columns, transposed orientation.
                # Both token halves share each weight chunk (one LDW, 2 MMs).
                ccs = range(12) if tq == 0 else range(6, 12)
                for cc in ccs:
                    pj = pap.tile([128, 1024], F32, tag="pj")
                    for kc in range(6):
                        for th in range(2):
                            nc.tensor.matmul(
                                pj[:, th * 512 : (th + 1) * 512],
                                wqkv_sb[:, kc, cc * 128 : (cc + 1) * 128],
                                xq[:, kc, th * 512 : (th + 1) * 512],
                                start=(kc == 0),
                                stop=(kc == 5),
                            )
                    if cc < 6:
                        # split into per-head halves of qpad; pad stays 0
                        nc.vector.tensor_copy(
                            qpad[0:64, cc, 0, :], pj[0:64, :]
                        )
                        nc.vector.tensor_copy(
                            qpad[64:128, cc, 1, :], pj[64:128, :]
                        )
                    else:
                        nc.vector.tensor_copy(
                            kT_sb[:, cc - 6, tq0 : tq0 + NT], pj[:]
                        )

                # v columns (token-major); one xq LDW feeds both col halves
                for tcn in range(8):
                    kcn = tq * 8 + tcn  # global 128-token (=key) chunk
                    pj = pap.tile([128, 768], F32, tag="pjv")
                    for kc in range(6):
                        for c0, c1 in ((0, 512), (512, 768)):
                            nc.tensor.matmul(
                                pj[:, c0:c1],
                                xq[:, kc, tcn * 128 : (tcn + 1) * 128],
                                wqkv_sb[:, kc, 2 * C + c0 : 2 * C + c1],
                                start=(kc == 0),
                                stop=(kc == 5),
                            )
                    nc.vector.tensor_copy(
                        v_sb[:, kcn, :, 0:D],
                        pj[:].rearrange("p (h d) -> p h d", d=D),
                    )

        # phase D weights: needed only at the end
        nc.sync.dma_start(bproj_sb[:], bproj_e[:])
        nc.gpsimd.partition_broadcast(bproj_bc[:], bproj_sb[:])
        for cc in range(6):
            nc.sync.dma_start(
                wproj_sb[:, cc, :], wproj_e[cc * 128 : (cc + 1) * 128, :]
            )

        # ================= phase C: attention =================
        # Per kc: one kT LDW + 4 K=128 score MMs (zero-padded q), 2 exps
        # on ACT, 4 K=128 AV MMs two iterations behind (et pool holds the
        # slack so PE never waits on ACT).
        # PSUM: sc 2x[128,1024] (2 banks each) + av 4x[65,512] = 8 banks.
        with (
            tc.tile_pool(name="et_pool", bufs=12) as etp,
            tc.tile_pool(name="sc_pool", bufs=2, space="PSUM") as scp,
            tc.tile_pool(name="av_pool", bufs=4, space="PSUM") as avp,
            tc.tile_pool(name="small", bufs=4) as smp,
        ):
            for hp in range(6):
                avs = []
                for i in range(4):
                    avt = avp.tile([65, 512], F32, tag="av", name=f"av_{hp}_{i}")
                    avs.append(avt)

                # AV for kc runs 2 iterations behind its exp
                def do_av(kc, e0, e1):
                    for hd, et in ((0, e0), (1, e1)):
                        for qh in range(2):
                            jsl = slice(qh * 512, (qh + 1) * 512)
                            nc.tensor.matmul(
                                avs[2 * hd + qh][:],
                                v_sb[:, kc, 2 * hp + hd, :],
                                et[:, jsl],
                                start=(kc == 0),
                                stop=(kc == KC - 1),
                            )

                pend = []
                for kc in range(KC):
                    ksl = slice(kc * 128, (kc + 1) * 128)
                    sc0 = scp.tile([128, 1024], F32, tag="sc")
                    sc1 = scp.tile([128, 1024], F32, tag="sc")
                    # full-K matmuls: one LDW (kT chunk) serves all four;
                    # the zero pad in qpad kills the cross-head terms
                    for qh in range(2):
                        qsl = slice(qh * 512, (qh + 1) * 512)
                        nc.tensor.matmul(
                            sc0[:, qsl],
                            kT_sb[:, hp, ksl],
                            qpad[:, hp, 0, qsl],
                            start=True,
                            stop=True,
                        )
                    for qh in range(2):
                        qsl = slice(qh * 512, (qh + 1) * 512)
                        nc.tensor.matmul(
                            sc1[:, qsl],
                            kT_sb[:, hp, ksl],
                            qpad[:, hp, 1, qsl],
                            start=True,
                            stop=True,
                        )
                    e0 = etp.tile([128, 1024], BF16, tag="et")
                    e1 = etp.tile([128, 1024], BF16, tag="et")
                    nc.scalar.activation(e0[:], sc0[:], EXP, scale=SCALE)
                    nc.scalar.activation(e1[:], sc1[:], EXP, scale=SCALE)
                    pend.append((kc, e0, e1))
                    if len(pend) > 1:
                        do_av(*pend.pop(0))
                for p in pend:
                    do_av(*p)

                # evacuate all four av banks FIRST: the next hp's AV
                # matmuls reuse these PSUM slots, so every normalization
                # op queued on DVE before the last copy delays them
                av_sbs = []
                for i, av in enumerate(avs):
                    av_sb = smp.tile(
                        [65, 512], F32, tag="av_sb", name=f"av_sb_{hp}_{i}"
                    )
                    nc.vector.tensor_copy(av_sb[:], av[:])
                    av_sbs.append(av_sb)
                # normalization chains, qh0 halves (both heads) first so
                # the tail's output projection can start on column block 0
                for i in (0, 2, 1, 3):
                    hd, qh = i // 2, i % 2
                    qsl = slice(qh * 512, (qh + 1) * 512)
                    av_sb = av_sbs[i]
                    # relocate denominator row to partition 0 (1-partition
                    # DVE copies cross quadrants; the custom approx-recip op
                    # needs all operands co-resident at partition 0)
                    den = smp.tile([1, 512], F32, tag="den")
                    nc.vector.tensor_copy(den[:], av_sb[64:65, :])
                    rec = smp.tile([1, 512], F32, tag="rec")
                    nc.vector.reciprocal_approx_fast(rec[:], den[:])
                    bc = smp.tile([64, 512], F32, tag="bc")
                    nc.gpsimd.partition_broadcast(bc[:], rec[:])
                    if hd == 0:
                        nc.vector.tensor_tensor(
                            out=tokT[hp][0:64, qsl],
                            in0=av_sb[0:64, :],
                            in1=bc[:],
                            op=MUL,
                        )
                    else:
                        tmp = smp.tile([64, 512], BF16, tag="tmp")
                        nc.vector.tensor_tensor(
                            out=tmp[:], in0=av_sb[0:64, :], in1=bc[:], op=MUL
                        )
                        # partition-shifting copy (base 0 -> 64) via DMA
                        nc.sync.dma_start(tokT[hp][64:128, qsl], tmp[:])

        # ================= phase D: output projection =================
        with (
            tc.tile_pool(name="pd_psum", bufs=4, space="PSUM") as pdp,
            tc.tile_pool(name="pd_sbuf", bufs=4) as pds,
        ):
            for tcn in range(8):
                pj = pdp.tile([128, 768], F32, tag="pd")
                for cc in range(6):
                    for c0, c1 in ((0, 512), (512, 768)):
                        nc.tensor.matmul(
                            pj[:, c0:c1],
                            tokT[cc][:, tcn * 128 : (tcn + 1) * 128],
                            wproj_sb[:, cc, c0:c1],
                            start=(cc == 0),
                            stop=(cc == 5),
                        )
                ot = pds.tile([128, 768], F32, tag="ot")
                nc.vector.tensor_tensor(
                    out=ot[:], in0=pj[:], in1=bproj_bc[:], op=ADD
                )
                nc.sync.dma_start(out_e[tcn * 128 : (tcn + 1) * 128, :], ot[:])


_CACHE = {}


def _get_graph():
    if "nc" not in _CACHE:
        _CACHE["nc"] = build_graph()
    return _CACHE["nc"]


def make_in_maps(x, W_qkv, W_proj, b_proj):
    x = np.asarray(x, dtype=np.float32)
    W_qkv = np.asarray(W_qkv, dtype=np.float32).astype(ml_dtypes.bfloat16)
    W_proj = np.asarray(W_proj, dtype=np.float32).astype(ml_dtypes.bfloat16)
    b_proj = np.asarray(b_proj, dtype=np.float32).reshape(1, C)
    W_qkv = np.ascontiguousarray(W_qkv)
    W_proj = np.ascontiguousarray(W_proj)
    in_maps = []
    for c in range(NCORES):
        bb, r0 = c // 4, (c % 4) * NT
        idx = np.r_[r0 : r0 + NT, 0:r0, r0 + NT : N]
        xT = np.ascontiguousarray(
            x[bb][idx].T.astype(ml_dtypes.bfloat16)
        )  # own tokens first
        in_maps.append(
            {
                "xT": xT,
                "Wqkv": W_qkv,
                "Wproj": W_proj,
                "bproj": b_proj,
            }
        )
    return in_maps


def run(x, W_qkv, W_proj, b_proj, trace=False):
    nc = _get_graph()
    in_maps = make_in_maps(x, W_qkv, W_proj, b_proj)
    res = run_bass_kernel_spmd(
        nc, in_maps, core_ids=list(range(NCORES)), trace=trace
    )
    out = np.zeros((B, N, C), dtype=np.float32)
    for c in range(NCORES):
        bb, r0 = c // 4, (c % 4) * NT
        out[bb, r0 : r0 + NT, :] = res.results[c]["out"]
    return out, res


def kernel(x, W_qkv, W_proj, b_proj):
    out, _ = run(x, W_qkv, W_proj, b_proj, trace=False)
    return out


# revision 33
# speedup vs baseline: 1.0241x; 1.0002x over previous
"""Trainium2 Bass kernel: 12-head self-attention block (B=2, N=4096, C=768).

Sharding: token-parallel over the 8192 (batch, token) rows. Core c (0..7)
handles batch c//4, query rows [(c%4)*1024, (c%4+1)*1024). Every core
redundantly computes K/V for its whole batch (zero collectives); the host
rotates each core's token order so its own 1024 query tokens come first
(attention is permutation-invariant over keys).

Phase C is ACT-bound: softmax exponentiates 50M scores per core; each
[128,1024] ACTIVATE measures ~1005ns -> ~386us floor for 384 tiles, and
the phase runs within ~1.5us of it (791ns total ACT stalls). The PE
must stay under that budget per key-chunk. Structural choices:

  * Scores are computed with K=128 matmuls against a zero-padded qT
    (qpad slot h0 has partitions 64:128 zeroed, slot h1 has 0:64 zeroed;
    lhsT is the full 128-partition kT chunk, so the cross-head contraction
    terms multiply by zero). K=64 matmuls would put the PE in 64x128
    tiling mode, and switching tiling modes between score and AV matmuls
    (K=128) drains the PE array -- measured ~90ns per matmul extra. With
    padding everything stays in 128x128 mode and one LDWEIGHTS per key
    chunk feeds all 4 score matmuls. Stream time is N cycles regardless
    of K, so the padded contraction costs nothing.
  * v_aug carries a ones column so the AV matmul emits softmax
    denominators for free (row 64); reciprocal_approx_fast (input
    relocated to partition 0) feeds a gpsimd broadcast and one DVE
    multiply per tile.
  * At each head-pair boundary all four av PSUM banks are evacuated
    to SBUF before any normalization op is queued on DVE: the next
    hp's AV matmuls reuse those banks, and interleaving the copies
    with the recip chains was measured to gate them ~6us late
    (1us ACT stall surfacing at kc3 of every hp).

All staging in SBUF, bf16 everywhere. fp8 was tested and rejected for
BOTH V (2.1e-2 rel err) and Q/K projections (9.9e-2 on hw, 9.6e-2 in a
numpy sim -- the softmax here is peaky, so score noise does not average
out). Cross-core K/V all-gather was measured and rejected: each
collective_compute on this axon harness has a ~60us entry/exit floor
(only ~20us wire time per 6.3MB chunk), so a pipelined exchange cannot
beat recomputation. Dense back-to-back matmul streaming keeps the PE
HAM clock gate at 2.4 GHz; prologue DMAs are interleaved (wqkv/x chunk
pairs) and memsets run on gpsimd so the first matmul issues at ~13us.

Device pipeline per core:
  phase A (per 1024-token quarter): qT/kT[col,t] = Wqkv[:, :1536].T @ x^T
           (q only for quarter 0 -> qpad halves; k -> kT_sb), v[t,(h,d)] =
           x @ Wqkv[:, 1536:] -> v_sb[t, kc, h, 0:64], ones at col 64.
  phase C: per head-pair hp, key-chunk kc: scoresT[key,q] = kT.T @ qpad_h,
           eT = exp(SCALE*sT) on ACT, av[0:65] += v_aug.T @ eT (one
           iteration behind its exp; et pool holds the slack).
  phase D: out[t,c] = tokensT.T @ Wproj + bproj.
"""

import sys

import numpy as np

try:
    import concourse  # noqa: F401
except ImportError:  # pragma: no cover
    sys.path.insert(0, "/opt/trn_rl_repo")

import ml_dtypes

import concourse.bass as bass  # noqa: F401
import concourse.mybir as mybir
import concourse.tile as tile
from concourse import bacc
from concourse.bass_utils import run_bass_kernel_spmd

B, N, C = 2, 4096, 768
H, D = 12, 64
NT = 1024  # query tokens per core
SCALE = float(D) ** -0.5
NCORES = 8
KC = N // 128  # 32 key chunks per batch
VW = D + 1  # v_aug row width per head: [v(64), ones]

F32 = mybir.dt.float32
BF16 = mybir.dt.bfloat16
EXP = mybir.ActivationFunctionType.Exp
MUL = mybir.AluOpType.mult
ADD = mybir.AluOpType.add


def build_graph():
    nc = bacc.Bacc(
        "TRN2", target_bir_lowering=False, debug=False, num_devices=NCORES
    )

    xT_e = nc.declare_dram_parameter("xT", [C, N], BF16, isOutput=False)
    wqkv_e = nc.declare_dram_parameter("Wqkv", [C, 3 * C], BF16, isOutput=False)
    wproj_e = nc.declare_dram_parameter("Wproj", [C, C], BF16, isOutput=False)
    bproj_e = nc.declare_dram_parameter("bproj", [1, C], F32, isOutput=False)
    out_e = nc.declare_dram_parameter("out", [NT, C], F32, isOutput=True)

    with tile.TileContext(nc) as tc:
        _build_body(nc, tc, xT_e, wqkv_e, wproj_e, bproj_e, out_e)
    nc.finalize()
    return nc


def _build_body(nc, tc, xT_e, wqkv_e, wproj_e, bproj_e, out_e):
    with tc.tile_pool(name="persist", bufs=1) as persist:
        # ---- persistent SBUF ----
        # qpad[:, hp, h, :]: head h of pair hp at partitions h*64:(h+1)*64,
        # other 64 partitions stay zero (enables K=128 score matmuls).
        qpad = persist.tile([128, 6, 2, NT], BF16, tag="qpad")
        kT_sb = persist.tile([128, 6, N], BF16, tag="kT")
        # v_aug[token_part, key_chunk, head, 0:64]=v, [.,.,.,64]=1.0
        v_sb = persist.tile([128, KC, H, VW], BF16, tag="v")
        tokT = [
            persist.tile([128, NT], BF16, tag=f"tokT{i}", name=f"tokT{i}")
            for i in range(6)
        ]
        # phase D weights, DMA'd off the prologue critical path
        wproj_sb = persist.tile([128, 6, C], BF16, tag="wproj")
        bproj_sb = persist.tile([1, C], F32, tag="bproj")
        bproj_bc = persist.tile([128, C], F32, tag="bproj_bc")

        # memsets on gpsimd: keeps the DVE queue free for phase A copies
        # (only the pad halves of qpad need zeroing; phase A writes the rest)
        nc.gpsimd.memset(qpad[64:128, :, 0, :], 0.0)
        nc.gpsimd.memset(qpad[0:64, :, 1, :], 0.0)
        # ones column of v_aug only (strided memset); v writes fill 0:64
        nc.gpsimd.memset(v_sb[:, :, :, D : D + 1], 1.0)

        # preload the exp ACT table set during phase A so phase C's first
        # activation doesn't pay the ~2.7us ACT_TABLE_LOAD + drain
        warm = persist.tile([1, 16], F32, tag="warm")
        nc.vector.memset(warm[:], 0.0)
        nc.scalar.activation(warm[:], warm[:], EXP, scale=1.0)

        # ================= phase A: qkv projection =================
        with (
            tc.tile_pool(name="pa_w", bufs=1) as paw,
            tc.tile_pool(name="pa_x", bufs=2) as pax,
            tc.tile_pool(name="pa_psum", bufs=2, space="PSUM") as pap,
        ):
            wqkv_sb = paw.tile([128, 6, 3 * C], BF16, tag="wqkv")
            xq0 = pax.tile([128, 6, NT], BF16, tag="xq")
            # interleave weight/activation chunk DMAs so the first matmul's
            # operands (wqkv kc0 + xq kc0) land first
            for kc in range(6):
                nc.sync.dma_start(
                    wqkv_sb[:, kc, :], wqkv_e[kc * 128 : (kc + 1) * 128, :]
                )
                nc.sync.dma_start(
                    xq0[:, kc, :], xT_e[kc * 128 : (kc + 1) * 128, 0:NT]
                )

            for tq in range(4):  # 1024-token quarters
                tq0 = tq * NT
                if tq == 0:
                    xq = xq0
                else:
                    xq = pax.tile([128, 6, NT], BF16, tag="xq")
                    for kc in range(6):
                        nc.sync.dma_start(
                            xq[:, kc, :],
                            xT_e[kc * 128 : (kc + 1) * 128, tq0 : tq0 + NT],
                        )

                # q (quarter 0 only) + k columns, transposed orientation.
                # Both token halves share each weight chunk (one LDW, 2 MMs).
                ccs = range(12) if tq == 0 else range(6, 12)
                for cc in ccs:
                    pj = pap.tile([128, 1024], F32, tag="pj")
                    for kc in range(6):
                        for th in range(2):
                            nc.tensor.matmul(
                                pj[:, th * 512 : (th + 1) * 512],
                                wqkv_sb[:, kc, cc * 128 : (cc + 1) * 128],
                                xq[:, kc, th * 512 : (th + 1) * 512],
                                start=(kc == 0),
                                stop=(kc == 5),
                            )
                    if cc < 6:
                        # split into per-head halves of qpad; pad stays 0
                        nc.vector.tensor_copy(
                            qpad[0:64, cc, 0, :], pj[0:64, :]
                        )
                        nc.vector.tensor_copy(
                            qpad[64:128, cc, 1, :], pj[64:128, :]
                        )
                    else:
                        nc.vector.tensor_copy(
                            kT_sb[:, cc - 6, tq0 : tq0 + NT], pj[:]
                        )

                # v columns (token-major); one xq LDW feeds both col halves
                for tcn in range(8):
                    kcn = tq * 8 + tcn  # global 128-token (=key) chunk
                    pj = pap.tile([128, 768], F32, tag="pjv")
                    for kc in range(6):
                        for c0, c1 in ((0, 512), (512, 768)):
                            nc.tensor.matmul(
                                pj[:, c0:c1],
                                xq[:, kc, tcn * 128 : (tcn + 1) * 128],
                                wqkv_sb[:, kc, 2 * C + c0 : 2 * C + c1],
                                start=(kc == 0),
                                stop=(kc == 5),
                            )
                    nc.vector.tensor_copy(
                        v_sb[:, kcn, :, 0:D],
                        pj[:].rearrange("p (h d) -> p h d", d=D),
                    )

        # phase D weights: needed only at the end
        nc.sync.dma_start(bproj_sb[:], bproj_e[:])
        nc.gpsimd.partition_broadcast(bproj_bc[:], bproj_sb[:])
        for cc in range(6):
            nc.sync.dma_start(
                wproj_sb[:, cc, :], wproj_e[cc * 128 : (cc + 1) * 128, :]
            )

        # ================= phase C: attention =================
        # Per kc: one kT LDW + 4 K=128 score MMs (zero-padded q), 2 exps
        # on ACT, 4 K=128 AV MMs two iterations behind (et pool holds the
        # slack so PE never waits on ACT).
        # PSUM: sc 2x[128,1024] (2 banks each) + av 4x[65,512] = 8 banks.
        with (
            tc.tile_pool(name="et_pool", bufs=12) as etp,
            tc.tile_pool(name="sc_pool", bufs=2, space="PSUM") as scp,
            tc.tile_pool(name="av_pool", bufs=4, space="PSUM") as avp,
            tc.tile_pool(name="small", bufs=4) as smp,
        ):
            for hp in range(6):
                avs = []
                for i in range(4):
                    avt = avp.tile([65, 512], F32, tag="av", name=f"av_{hp}_{i}")
                    avs.append(avt)

                # AV for kc runs 2 iterations behind its exp
                def do_av(kc, e0, e1):
                    for hd, et in ((0, e0), (1, e1)):
                        for qh in range(2):
                            jsl = slice(qh * 512, (qh + 1) * 512)
                            nc.tensor.matmul(
                                avs[2 * hd + qh][:],
                                v_sb[:, kc, 2 * hp + hd, :],
                                et[:, jsl],
                                start=(kc == 0),
                                stop=(kc == KC - 1),
                            )

                pend = []
                for kc in range(KC):
                    ksl = slice(kc * 128, (kc + 1) * 128)
                    sc0 = scp.tile([128, 1024], F32, tag="sc")
                    sc1 = scp.tile([128, 1024], F32, tag="sc")
                    # full-K matmuls: one LDW (kT chunk) serves all four;
                    # the zero pad in qpad kills the cross-head terms
                    for qh in range(2):
                        qsl = slice(qh * 512, (qh + 1) * 512)
                        nc.tensor.matmul(
                            sc0[:, qsl],
                            kT_sb[:, hp, ksl],
                            qpad[:, hp, 0, qsl],
                            start=True,
                            stop=True,
                        )
                    for qh in range(2):
                        qsl = slice(qh * 512, (qh + 1) * 512)
                        nc.tensor.matmul(
                            sc1[:, qsl],
                            kT_sb[:, hp, ksl],
                            qpad[:, hp, 1, qsl],
                            start=True,
                            stop=True,
                        )
                    e0 = etp.tile([128, 1024], BF16, tag="et")
                    e1 = etp.tile([128, 1024], BF16, tag="et")
                    nc.scalar.activation(e0[:], sc0[:], EXP, scale=SCALE)
                    nc.scalar.activation(e1[:], sc1[:], EXP, scale=SCALE)
                    pend.append((kc, e0, e1))
                    if len(pend) > 1:
                        do_av(*pend.pop(0))
                for p in pend:
                    do_av(*p)

                # evacuate all four av banks FIRST: the next hp's AV
                # matmuls reuse these PSUM slots, so every normalization
                # op queued on DVE before the last copy delays them
                av_sbs = []
                for i, av in enumerate(avs):
                    av_sb = smp.tile(
                        [65, 512], F32, tag="av_sb", name=f"av_sb_{hp}_{i}"
                    )
                    nc.vector.tensor_copy(av_sb[:], av[:])
                    av_sbs.append(av_sb)
                # normalization chains, qh0 halves (both heads) first so
                # the tail's output projection can start on column block 0
                for i in (0, 2, 1, 3):
                    hd, qh = i // 2, i % 2
                    qsl = slice(qh * 512, (qh + 1) * 512)
                    av_sb = av_sbs[i]
                    # relocate denominator row to partition 0 (1-partition
                    # DVE copies cross quadrants; the custom approx-recip op
                    # needs all operands co-resident at partition 0)
                    den = smp.tile([1, 512], F32, tag="den")
                    nc.vector.tensor_copy(den[:], av_sb[64:65, :])
                    rec = smp.tile([1, 512], F32, tag="rec")
                    nc.vector.reciprocal_approx_fast(rec[:], den[:])
                    bc = smp.tile([64, 512], F32, tag="bc")
                    nc.gpsimd.partition_broadcast(bc[:], rec[:])
                    if hd == 0:
                        nc.vector.tensor_tensor(
                            out=tokT[hp][0:64, qsl],
                            in0=av_sb[0:64, :],
                            in1=bc[:],
                            op=MUL,
                        )
                    else:
                        tmp = smp.tile([64, 512], BF16, tag="tmp")
                        nc.vector.tensor_tensor(
                            out=tmp[:], in0=av_sb[0:64, :], in1=bc[:], op=MUL
                        )
                        # partition-shifting copy (base 0 -> 64) via DMA
                        nc.sync.dma_start(tokT[hp][64:128, qsl], tmp[:])

        # ================= phase D: output projection =================
        with (
            tc.tile_pool(name="pd_psum", bufs=4, space="PSUM") as pdp,
            tc.tile_pool(name="pd_sbuf", bufs=4) as pds,
        ):
            for tcn in range(8):
                pj = pdp.tile([128, 768], F32, tag="pd")
                for cc in range(6):
                    for c0, c1 in ((0, 512), (512, 768)):
                        nc.tensor.matmul(
                            pj[:, c0:c1],
                            tokT[cc][:, tcn * 128 : (tcn + 1) * 128],
                            wproj_sb[:, cc, c0:c1],
                            start=(cc == 0),
                            stop=(cc == 5),
                        )
                ot = pds.tile([128, 768], F32, tag="ot")
                nc.vector.tensor_tensor(
                    out=ot[:], in0=pj[:], in1=bproj_bc[:], op=ADD
                )
                nc.sync.dma_start(out_e[tcn * 128 : (tcn + 1) * 128, :], ot[:])


_CACHE = {}


def _get_graph():
    if "nc" not in _CACHE:
        _CACHE["nc"] = build_graph()
    return _CACHE["nc"]


def make_in_maps(x, W_qkv, W_proj, b_proj):
    x = np.asarray(x, dtype=np.float32)
    W_qkv = np.asarray(W_qkv, dtype=np.float32).astype(ml_dtypes.bfloat16)
    W_proj = np.asarray(W_proj, dtype=np.float32).astype(ml_dtypes.bfloat16)
    b_proj = np.asarray(b_proj, dtype=np.float32).reshape(1, C)
    W_qkv = np.ascontiguousarray(W_qkv)
    W_proj = np.ascontiguousarray(W_proj)
    in_maps = []
    for c in range(NCORES):
        bb, r0 = c // 4, (c % 4) * NT
        idx = np.r_[r0 : r0 + NT, 0:r0, r0 + NT : N]
        xT = np.ascontiguousarray(
            x[bb][idx].T.astype(ml_dtypes.bfloat16)
        )  # own tokens first
        in_maps.append(
            {
                "xT": xT,
                "Wqkv": W_qkv,
                "Wproj": W_proj,
                "bproj": b_proj,
            }
        )
    return in_maps


def run(x, W_qkv, W_proj, b_proj, trace=False):
    nc = _get_graph()
    in_maps = make_in_maps(x, W_qkv, W_proj, b_proj)
    res = run_bass_kernel_spmd(
        nc, in_maps, core_ids=list(range(NCORES)), trace=trace
    )
    out = np.zeros((B, N, C), dtype=np.float32)
    for c in range(NCORES):
        bb, r0 = c // 4, (c % 4) * NT
        out[bb, r0 : r0 + NT, :] = res.results[c]["out"]
    return out, res


def kernel(x, W_qkv, W_proj, b_proj):
    out, _ = run(x, W_qkv, W_proj, b_proj, trace=False)
    return out
